# revision 7
# baseline (speedup 1.0000x reference)
"""Trainium2 Bass kernel for nn_MultiHeadFastSelfAttention.

B=4, S=8192, D=1024, H=16, DH=64. 8 NeuronCores; core 2*b+j handles batch b,
sequence rows [j*4096, (j+1)*4096). One all-reduce (pairs) per softmax pooling.

Algorithm (validated in mock.py): the two pooling phases never materialize
mixed_q / mixed_k. With Wqqa = Wq @ Wqa folded on host:
  scores1 = x @ Wqqa;  e1 = exp(scale*scores1 + scale*bqqa + mask)
  P1 = e1^T @ x, se1 = sum(e1)            -> AllReduce over the pair
  pq_full[16,1024] = (P1 @ Wq)/se1 + blkdiag(bq);  pq = blockdiag extract
  W2 = Wk @ (pq*Wka)  (via W2^T = sum_c (pq*Wka)_c^T @ WkT_c), b2 = (bk*pq)@Wka + bka
  scores2 = x @ W2;  e2, P2, se2          -> AllReduce
  pk_full = ((P2 @ Wk)/se2 + blkdiag(bk)) * pq_full; pk = extract
  M3 = pk[:,None]*Wt + I;  W3 = Wq @ M3;  b3 = bq @ M3 + bt
  out = x @ W3 + b3
Device tensors live in transposed [D, rows] layout; host pre/post transposes.
Matmuls run as float32r (full-rate fp32); the e^T @ x pooling accumulation
uses bf16 inputs.
"""

import numpy as np
import ml_dtypes
from contextlib import ExitStack

import concourse.bass as bass
import concourse.tile as tile
from concourse import bacc, mybir
from concourse.bass_utils import run_bass_kernel_spmd
from concourse.masks import make_identity

F32 = mybir.dt.float32
F32R = mybir.dt.float32r
BF16 = mybir.dt.bfloat16
AF = mybir.ActivationFunctionType

B, S, D, H = 4, 8192, 1024, 16
DH = D // H
SCALE = 0.125  # 1/sqrt(64)
CH = 8  # 128-partition chunks across D





def build_program(rows=4096, r_tile=512, num_cores=8):
    """Build the SPMD Bass program (identical on every core)."""
    nt = rows // r_tile
    nsub = rows // 128
    nu = r_tile // 128
    rg = [[2 * i, 2 * i + 1] for i in range(num_cores // 2)]

    nc = bacc.Bacc(
        "TRN2",
        target_bir_lowering=False,
        debug=False,
        enable_asserts=True,
        num_devices=num_cores,
    )

    # ---- kernel I/O ----
    xT = nc.dram_tensor("xT", [D, rows], F32R, kind="ExternalInput").ap()
    xnat = nc.dram_tensor("xnat", [rows, D], BF16, kind="ExternalInput").ap()
    mask8 = nc.dram_tensor("mask8", [1, rows], F32R, kind="ExternalInput").ap()
    wqqa = nc.dram_tensor("wqqa", [D, H], F32R, kind="ExternalInput").ap()
    bqqa8T = nc.dram_tensor("bqqa8T", [H, 1], F32, kind="ExternalInput").ap()
    wq = nc.dram_tensor("wq", [D, D], F32R, kind="ExternalInput").ap()
    wkT = nc.dram_tensor("wkT", [D, D], F32R, kind="ExternalInput").ap()
    wk = nc.dram_tensor("wk", [D, D], F32R, kind="ExternalInput").ap()
    wt = nc.dram_tensor("wt", [D, D], F32, kind="ExternalInput").ap()
    wqt_blk = nc.dram_tensor("wqt_blk", [CH, CH, 128, 128], F32R, kind="ExternalInput").ap()
    wka = nc.dram_tensor("wka", [D, H], F32, kind="ExternalInput").ap()
    bka8T = nc.dram_tensor("bka8T", [H, 1], F32, kind="ExternalInput").ap()
    bk8_ch = nc.dram_tensor("bk8_ch", [128, CH], F32, kind="ExternalInput").ap()
    bq_ch = nc.dram_tensor("bq_ch", [128, CH], F32, kind="ExternalInput").ap()
    bqbt_ch = nc.dram_tensor("bqbt_ch", [128, CH], F32, kind="ExternalInput").ap()
    bq_blk = nc.dram_tensor("bq_blk", [H, D], F32, kind="ExternalInput").ap()
    bk_blk = nc.dram_tensor("bk_blk", [H, D], F32, kind="ExternalInput").ap()
    ones16_d = nc.dram_tensor("ones16_d", [1, H], F32R, kind="ExternalInput").ap()
    outT = nc.dram_tensor("outT", [D, rows], F32, kind="ExternalOutput").ap()

    xT_re = xT.rearrange("(c p) n -> p c n", p=128)
    xnat_re = xnat.rearrange("(g p) d -> p g d", p=128)
    wq_re = wq.rearrange("(c p) n -> p c n", p=128)
    wkT_re = wkT.rearrange("(c p) n -> p c n", p=128)
    wk_re = wk.rearrange("(c p) n -> p c n", p=128)
    wt_re = wt.rearrange("(c p) n -> p c n", p=128)
    outT_re = outT.rearrange("(c p) n -> p c n", p=128)

    with tile.TileContext(nc) as tc, ExitStack() as ctx:
        const = ctx.enter_context(tc.tile_pool(name="const", bufs=1))
        state = ctx.enter_context(tc.tile_pool(name="state", bufs=1))
        xt_pool = ctx.enter_context(tc.tile_pool(name="xt", bufs=2))
        wstream = ctx.enter_context(tc.tile_pool(name="wstream", bufs=2))
        small = ctx.enter_context(tc.tile_pool(name="small", bufs=3))
        ps_small = ctx.enter_context(tc.tile_pool(name="ps_small", bufs=2, space="PSUM"))
        dram = ctx.enter_context(tc.tile_pool(name="dram", bufs=1, space="DRAM"))

        # ---- constants ----
        wqqa_sb = const.tile([128, CH, H], F32R)
        nc.sync.dma_start(out=wqqa_sb, in_=wqqa.rearrange("(c p) h -> p c h", p=128))
        wka_sb = const.tile([128, CH, H], F32)
        nc.sync.dma_start(out=wka_sb, in_=wka.rearrange("(c p) h -> p c h", p=128))
        bqqa_sb = const.tile([H, 1], F32)
        nc.sync.dma_start(out=bqqa_sb, in_=bqqa8T)
        bka_sb = const.tile([H, 1], F32)
        nc.sync.dma_start(out=bka_sb, in_=bka8T)
        bk8_sb = const.tile([128, CH], F32)
        nc.sync.dma_start(out=bk8_sb, in_=bk8_ch)
        bq_sb = const.tile([128, CH], F32)
        nc.sync.dma_start(out=bq_sb, in_=bq_ch)
        bqbt_sb = const.tile([128, CH], F32)
        nc.sync.dma_start(out=bqbt_sb, in_=bqbt_ch)
        bqblk_sb = const.tile([H, D], F32)
        nc.sync.dma_start(out=bqblk_sb, in_=bq_blk)
        bkblk_sb = const.tile([H, D], F32)
        nc.sync.dma_start(out=bkblk_sb, in_=bk_blk)
        mask_sb = const.tile([1, rows], F32R)
        nc.sync.dma_start(out=mask_sb, in_=mask8)
        ident = const.tile([128, 128], F32)
        make_identity(nc, ident)
        ones16 = const.tile([1, H], F32R)
        nc.sync.dma_start(out=ones16, in_=ones16_d)

        # ---- cross-phase state ----
        p_acc = [state.tile([H, D], F32, name=f"p{i}acc") for i in (1, 2)]
        se_acc = [state.tile([H, 1], F32, name=f"se{i}acc") for i in (1, 2)]
        for t_ in p_acc + se_acc:
            nc.vector.memset(t_, 0.0)
        pg_sb = [state.tile([H, D + 1], F32, name=f"p{i}g") for i in (1, 2)]
        pgT = [state.tile([128, CH, H], F32R, name=f"p{i}gT") for i in (1, 2)]
        rse = [state.tile([H, 1], F32, name=f"rse{i}") for i in (1, 2)]
        pqfull = state.tile([H, D], F32)
        pkfull = state.tile([H, D], F32)
        pq_chk = state.tile([128, CH], F32)
        pk_chk = state.tile([128, CH], F32)
        w2T_sb = state.tile([H, D], F32)
        w2_sb = state.tile([128, CH, H], F32R)
        b2T_sb = state.tile([H, 1], F32)
        b3T_sb = state.tile([128, CH], F32)

        bounce_in = [dram.tile([H, D + 1], F32, name=f"cc_in{i}") for i in (1, 2)]
        bounce_out = [dram.tile([H, D + 1], F32, name=f"cc_out{i}") for i in (1, 2)]

        def extract_chunks(full, dest):
            """dest[:, c] (128 rows) = blockdiag values of full[16, 1024]."""
            for c in range(CH):
                pt = ps_small.tile([128, H], F32, name="pt")
                nc.tensor.transpose(pt, full[:, c * 128:(c + 1) * 128], ident[:H, :H])
                nc.vector.tensor_copy(dest[0:64, c:c + 1], pt[0:64, 2 * c:2 * c + 1])
                nc.vector.tensor_copy(dest[64:128, c:c + 1],
                                      pt[64:128, 2 * c + 1:2 * c + 2])

        # ============ phases 1 & 2 (scoped pools) ============
        with tc.tile_pool(name="xnatp", bufs=1) as xnatp, \
             tc.tile_pool(name="e", bufs=2) as e_pool, \
             tc.tile_pool(name="eT", bufs=3) as eT_pool, \
             tc.tile_pool(name="ps_score", bufs=2, space="PSUM") as ps_score, \
             tc.tile_pool(name="ps_acc", bufs=2, space="PSUM") as ps_acc:

            xnat_sb = xnatp.tile([128, nsub, D], BF16)
            nc.sync.dma_start(out=xnat_sb, in_=xnat_re)

            def score_phase(ph, w_sb, biasT):
                """scores = x @ W (+mask); e = Exp(0.125*s + biasT); accumulate
                se_acc[ph] and P = e^T @ x into p_acc[ph]."""
                for t in range(nt):
                    xt = xt_pool.tile([128, CH, r_tile], F32R, name="xt")
                    nc.sync.dma_start(out=xt,
                                      in_=xT_re[:, :, t * r_tile:(t + 1) * r_tile])
                    ps = ps_score.tile([H, r_tile], F32, name="ps_sc")
                    for c in range(CH):
                        nc.tensor.matmul(ps, w_sb[:, c, :], xt[:, c, :],
                                         start=(c == 0), stop=False)
                    nc.tensor.matmul(
                        ps, ones16,
                        mask_sb[:, t * r_tile:(t + 1) * r_tile],
                        start=False, stop=True)
                    e = e_pool.tile([H, r_tile], F32, name="e")
                    sep = small.tile([H, 1], F32, name="sep")
                    nc.scalar.activation(e, ps, AF.Exp, bias=biasT, scale=SCALE,
                                         accum_out=sep)
                    nc.vector.tensor_add(se_acc[ph], se_acc[ph], sep)
                    pp = ps_acc.tile([H, D], F32, name="pp")
                    for u in range(nu):
                        pt = ps_small.tile([128, H], F32, name="pt")
                        nc.tensor.transpose(pt, e[:, u * 128:(u + 1) * 128], ident[:H, :H])
                        eT = eT_pool.tile([128, H], BF16, name="eT")
                        nc.vector.tensor_copy(eT, pt)
                        g = t * nu + u
                        for hf in range(2):
                            nc.tensor.matmul(
                                pp[:, hf * 512:(hf + 1) * 512], eT,
                                xnat_sb[:, g, hf * 512:(hf + 1) * 512],
                                start=(u == 0), stop=(u == nu - 1),
                                skip_group_check=True)
                    nc.vector.tensor_add(p_acc[ph], p_acc[ph], pp)

            def all_reduce(ph):
                nc.sync.dma_start(out=bounce_in[ph][:, 0:D], in_=p_acc[ph])
                nc.sync.dma_start(out=bounce_in[ph][:, D:D + 1], in_=se_acc[ph])
                nc.gpsimd.collective_compute(
                    "AllReduce", mybir.AluOpType.add, replica_groups=rg,
                    ins=[bounce_in[ph].opt()], outs=[bounce_out[ph].opt()])
                nc.sync.dma_start(out=pg_sb[ph], in_=bounce_out[ph])
                nc.vector.reciprocal(rse[ph], pg_sb[ph][:, D:D + 1])
                for c in range(CH):
                    pt = ps_small.tile([128, H], F32, name="pt")
                    nc.tensor.transpose(pt, pg_sb[ph][:, c * 128:(c + 1) * 128],
                                        ident[:H, :H])
                    nc.vector.tensor_copy(pgT[ph][:, c, :], pt)

            def pooled_full(ph, w_re, dest):
                """dest[16, D] = (Pg @ W) * (1/se_g)."""
                ps = ps_acc.tile([H, D], F32, name="pp")
                for c in range(CH):
                    wc = wstream.tile([128, D], F32R, name="wc")
                    nc.sync.dma_start(out=wc, in_=w_re[:, c, :])
                    for hf in range(2):
                        nc.tensor.matmul(
                            ps[:, hf * 512:(hf + 1) * 512], pgT[ph][:, c, :],
                            wc[:, hf * 512:(hf + 1) * 512],
                            start=(c == 0), stop=(c == CH - 1),
                            skip_group_check=True)
                nc.scalar.activation(dest, ps, AF.Copy, scale=rse[ph])

            # ---------- phase 1 ----------
            score_phase(0, wqqa_sb, bqqa_sb)
            all_reduce(0)
            pooled_full(0, wq_re, pqfull)
            nc.vector.tensor_add(pqfull, pqfull, bqblk_sb)
            extract_chunks(pqfull, pq_chk)

            # ---- build W2 (lhsT chunks) and b2T ----
            ps2 = ps_acc.tile([H, D], F32, name="pp")
            a_list = []
            for c in range(CH):
                ac = small.tile([128, H], F32R, name=f"a{c}", bufs=1)
                nc.vector.tensor_scalar_mul(ac, wka_sb[:, c, :], pq_chk[:, c:c + 1])
                a_list.append(ac)
            for c in range(CH):
                wc = wstream.tile([128, D], F32R, name="wc")
                nc.sync.dma_start(out=wc, in_=wkT_re[:, c, :])
                for hf in range(2):
                    nc.tensor.matmul(
                        ps2[:, hf * 512:(hf + 1) * 512], a_list[c],
                        wc[:, hf * 512:(hf + 1) * 512],
                        start=(c == 0), stop=(c == CH - 1), skip_group_check=True)
            nc.vector.tensor_copy(w2T_sb, ps2)
            for c in range(CH):
                pt = ps_small.tile([128, H], F32, name="pt")
                nc.tensor.transpose(pt, w2T_sb[:, c * 128:(c + 1) * 128], ident[:H, :H])
                nc.vector.tensor_copy(w2_sb[:, c, :], pt)
            bkpq = small.tile([128, CH], F32, name="bkpq", bufs=1)
            nc.vector.tensor_mul(bkpq, bk8_sb, pq_chk)
            psb = ps_small.tile([1, H], F32, name="psb", tag="pt")
            for c in range(CH):
                nc.tensor.matmul(psb, bkpq[:, c:c + 1], wka_sb[:, c, :],
                                 start=(c == 0), stop=(c == CH - 1))
            b2row = small.tile([1, H], F32, name="b2row", bufs=1)
            nc.vector.tensor_copy(b2row, psb)
            ptb2 = ps_small.tile([H, 1], F32, name="ptb2", tag="pt")
            nc.tensor.transpose(ptb2, b2row, ident[:1, :1])
            nc.vector.tensor_add(b2T_sb, ptb2, bka_sb)

            # ---------- phase 2 ----------
            score_phase(1, w2_sb, b2T_sb)
            all_reduce(1)
            pooled_full(1, wk_re, pkfull)
            nc.vector.tensor_add(pkfull, pkfull, bkblk_sb)
            nc.vector.tensor_mul(pkfull, pkfull, pqfull)
            extract_chunks(pkfull, pk_chk)

        # ============ phase 3 ============
        with tc.tile_pool(name="m3p", bufs=1) as m3p, \
             tc.tile_pool(name="w3p", bufs=1) as w3p, \
             tc.tile_pool(name="wqtp", bufs=2) as wqtp, \
             tc.tile_pool(name="outp", bufs=3) as outp, \
             tc.tile_pool(name="ps_out", bufs=4, space="PSUM") as ps_out:
            m3 = m3p.tile([128, CH, D], F32R)
            bqpk = small.tile([128, CH], F32, name="bqpk", bufs=1)
            nc.vector.tensor_mul(bqpk, bq_sb, pk_chk)
            psb3 = ps_out.tile([1, D], F32, name="psb3", tag="b3", bufs=1)
            for c in range(CH):
                wc = wstream.tile([128, D], F32, name="wc")
                nc.sync.dma_start(out=wc, in_=wt_re[:, c, :])
                nc.vector.tensor_scalar_mul(m3[:, c, :], wc, pk_chk[:, c:c + 1])
                nc.vector.tensor_add(m3[:, c, c * 128:(c + 1) * 128],
                                     m3[:, c, c * 128:(c + 1) * 128], ident)
                for hf in range(2):
                    nc.tensor.matmul(psb3[:, hf * 512:(hf + 1) * 512],
                                     bqpk[:, c:c + 1],
                                     wc[:, hf * 512:(hf + 1) * 512],
                                     start=(c == 0), stop=(c == CH - 1),
                                     skip_group_check=True)
            w3 = w3p.tile([128, CH, D], F32R)
            for k in range(CH):
                wqtb = wqtp.tile([128, CH, 128], F32R, name="wqtb")
                nc.sync.dma_start(out=wqtb,
                                  in_=wqt_blk[k].rearrange("c p j -> p c j"))
                for hf in range(2):
                    pw = ps_out.tile([128, 512], F32, name="pw", tag="po")
                    for c in range(CH):
                        nc.tensor.matmul(
                            pw, wqtb[:, c, :],
                            m3[:, c, hf * 512:(hf + 1) * 512],
                            start=(c == 0), stop=(c == CH - 1))
                    nc.vector.tensor_copy(w3[:, k, hf * 512:(hf + 1) * 512], pw)
            # b3 = (bq*pk) @ Wt + bq + bt; chunk to [128, 8] via PE transposes
            b3row = small.tile([1, D], F32, name="b3row", bufs=1)
            nc.vector.tensor_copy(b3row, psb3)
            for m in range(CH):
                ptb = ps_small.tile([128, 1], F32, name="ptb", tag="pt")
                nc.tensor.transpose(ptb, b3row[:, m * 128:(m + 1) * 128],
                                    ident[:1, :1])
                nc.vector.tensor_add(b3T_sb[:, m:m + 1], ptb,
                                     bqbt_sb[:, m:m + 1])
            # main: outT tiles = W3-blocks^T @ xT tiles (+ b3T)
            for t in range(nt):
                xt = xt_pool.tile([128, CH, r_tile], F32R, name="xt")
                nc.sync.dma_start(out=xt,
                                  in_=xT_re[:, :, t * r_tile:(t + 1) * r_tile])
                for m in range(CH):
                    po = ps_out.tile([128, r_tile], F32, name="po", tag="po")
                    for k in range(CH):
                        nc.tensor.matmul(po, w3[:, k, m * 128:(m + 1) * 128],
                                         xt[:, k, :],
                                         start=(k == 0), stop=(k == CH - 1))
                    ot = outp.tile([128, r_tile], F32, name="ot")
                    nc.vector.tensor_scalar_add(ot, po, b3T_sb[:, m:m + 1])
                    nc.sync.dma_start(
                        out=outT_re[:, m, t * r_tile:(t + 1) * r_tile], in_=ot)

    nc.compile()
    return nc


# ---------------- host side ----------------

def _blkdiag(v):
    m = np.zeros((H, D), np.float32)
    for h in range(H):
        m[h, h * DH:(h + 1) * DH] = v[h * DH:(h + 1) * DH]
    return m


def make_in_maps(hidden_states, attention_mask, Wq, bq, Wqa, bqa, Wk, bk, Wka,
                 bka, Wt, bt, rows=S // 2, num_cores=8):
    hs = np.ascontiguousarray(np.asarray(hidden_states, np.float32))
    mask = np.asarray(attention_mask, np.float32)
    Wq, bq, Wqa, bqa = (np.asarray(a, np.float32) for a in (Wq, bq, Wqa, bqa))
    Wk, bk, Wka, bka = (np.asarray(a, np.float32) for a in (Wk, bk, Wka, bka))
    Wt, bt = np.asarray(Wt, np.float32), np.asarray(bt, np.float32)

    # wqt_blk[k][c][p][j] = WqT[128c+p, 128k+j] = Wq[128k+j, 128c+p]
    wqt_blk = np.ascontiguousarray(
        np.transpose(Wq.T.reshape(CH, 128, CH, 128), (2, 0, 1, 3)))
    shared = {
        "wqqa": np.ascontiguousarray(Wq @ Wqa),
        "bqqa8T": ((bq @ Wqa + bqa) * SCALE).reshape(H, 1),
        "wq": Wq,
        "wkT": np.ascontiguousarray(Wk.T),
        "wk": Wk,
        "wt": Wt,
        "wqt_blk": wqt_blk,
        "wka": np.ascontiguousarray(Wka),
        "bka8T": (bka * SCALE).reshape(H, 1),
        "bk8_ch": np.ascontiguousarray((bk * SCALE).reshape(CH, 128).T),
        "bq_ch": np.ascontiguousarray(bq.reshape(CH, 128).T),
        "bqbt_ch": np.ascontiguousarray((bq + bt).reshape(CH, 128).T),
        "bq_blk": _blkdiag(bq),
        "ones16_d": np.ones((1, H), np.float32),
        "bk_blk": _blkdiag(bk),
    }
    n_batches = max(1, num_cores // 2)
    in_maps = []
    for core in range(num_cores):
        b, j = core // 2, core % 2
        x = hs[b, j * rows:(j + 1) * rows, :]
        m = mask[b, 0, j * rows:(j + 1) * rows]
        im = dict(shared)
        im["xT"] = np.ascontiguousarray(x.T)
        im["xnat"] = np.ascontiguousarray(x).astype(ml_dtypes.bfloat16)
        im["mask8"] = np.ascontiguousarray((m * 8.0).reshape(1, rows))
        in_maps.append(im)
    return in_maps


_PROGRAM = None


def _get_program():
    global _PROGRAM
    if _PROGRAM is None:
        _PROGRAM = build_program(rows=S // 2, r_tile=512, num_cores=8)
    return _PROGRAM


def run_on_hw(in_maps, **kwargs):
    return run_bass_kernel_spmd(_get_program(), in_maps,
                                core_ids=list(range(8)), **kwargs)


def assemble_output(results, rows=S // 2):
    out = np.empty((B, S, D), np.float32)
    for core in range(8):
        b, j = core // 2, core % 2
        out[b, j * rows:(j + 1) * rows, :] = results[core]["outT"].T
    return out


def kernel(**inputs):
    in_maps = make_in_maps(**inputs)
    res = run_on_hw(in_maps)
    return assemble_output(res.results)
